# revision 7
# baseline (speedup 1.0000x reference)
"""Trainium2 Bass kernel for the LSQ-quantized BasicBlock (nn_BasicBlock_45011257262579).

Contract: kernel(**inputs) takes the FULL unsharded inputs from setup_inputs()
(x [32,128,56,56] plus weights/BN stats) and returns the FULL output
[32,128,56,56] float32. Internally shards batch 32 across 8 NeuronCores
(4 images per core), runs a Bass/Tile kernel per core via
run_bass_kernel_spmd, and reassembles.

Algorithm per core (channels C=128 = SBUF partitions):
  - 3x3 conv = 9 shifted 1x1 convs (matmuls) over a zero-padded [58,58] image.
  - Weights are pre-quantized to small integers on host:
        Wint = round(clip(W/a_w, -4, 3))  (exact in any dtype)
    Conv matmul runs in float32r (TF32-like, ~1 cyc/col) with a 2-split of
    the activations (hi = f32r(v), lo = f32r(v - hi)) accumulated in PSUM,
    giving fp32-grade precision at ~2.1 cyc/col.
  - Per-partial-sum LSQ quant: z = s_i * psum (s_i = a_w[i]/a_p), then
    k = clip(round(z), -4, 3). Implemented as:
        ACT:  t = Identity(s_i * psum + BIGC)    # fp32; BIGC=1.5*2^23 makes
                                                 # the fp32 add itself RNE-round z
        DVE:  u = (t - BIGC) max -4   -> bf16    # exact small ints
        DVE:  c = u min 3             -> bf16
        DVE:  K += c                             # bf16 accumulate (exact ints)
  - BN (fixed stats) folds to per-channel affine: y = relu(g1*K + h1) with
    g1 = a_p*inv, h1 = beta - mean*inv (host fp32, matches reference ops).
  - Layer 2 same; final out = relu(g2*K2 + h2 + x).
"""

import sys
import numpy as np

sys.path.insert(0, "/opt/trn_rl_repo")

_CACHE = {}

NBITS_QN, NBITS_QP = -4.0, 3.0
BIGC = float(np.float32(1.5 * 2 ** 23))  # 12582912.0
SHIFTS = [(0, 0), (1, 0), (2, 0), (0, 1), (1, 1), (2, 1), (0, 2), (1, 2), (2, 2)]


def _build(B_loc, Himg, Wimg, scales1, scales2, debug=False):
    """Build + compile the per-core Bass program. scales{1,2} are tuples of 9
    python floats baked as ACT immediates."""
    import concourse.bass as bass  # noqa: F401
    import concourse.mybir as mybir
    from concourse import tile, bacc

    f32 = mybir.dt.float32
    f32r = mybir.dt.float32r
    bf16 = mybir.dt.bfloat16
    AF = mybir.ActivationFunctionType
    OP = mybir.AluOpType

    Hp, Wp = Himg + 2, Wimg + 2          # padded
    NPIX = Himg * Wimg                   # interior pixels
    NPAD = Hp * Wp
    # chunking of output rows: ROWS_PER_CHUNK rows -> N = ROWS*W cols per matmul
    RPC = 7 if Himg % 7 == 0 else (Himg // 8 if Himg % 8 == 0 else 1)
    while Himg % RPC:
        RPC -= 1
    NCH = Himg // RPC                    # chunks per image
    CPG = 4 if NCH % 4 == 0 else (2 if NCH % 2 == 0 else 1)  # chunks per group
    NG = NCH // CPG                      # groups
    NCOL = RPC * Wimg                    # cols per chunk (<=512 for psum bank)
    assert NCOL <= 512
    NGRP = CPG * NCOL                    # cols per group

    nc = bacc.Bacc("TRN2", target_bir_lowering=False, debug=False, num_devices=8)

    x_d = nc.dram_tensor("x", [B_loc, 128, NPIX], f32, kind="ExternalInput")
    w1_d = nc.dram_tensor("w1", [9, 128, 128], f32, kind="ExternalInput")
    w2_d = nc.dram_tensor("w2", [9, 128, 128], f32, kind="ExternalInput")
    gh_d = nc.dram_tensor("gh", [128, 4], f32, kind="ExternalInput")
    out_d = nc.dram_tensor("out", [B_loc, 128, NPIX], f32, kind="ExternalOutput")
    if debug:
        k1_d = nc.dram_tensor("k1", [B_loc, 128, NPIX], f32, kind="ExternalOutput")
        y_d = nc.dram_tensor("y", [B_loc, 128, NPAD], f32, kind="ExternalOutput")

    with tile.TileContext(nc) as tc:
        with tc.tile_pool(name="const", bufs=1) as cpool, \
             tc.tile_pool(name="img", bufs=1) as ipool, \
             tc.tile_pool(name="k1p", bufs=2) as kpool, \
             tc.tile_pool(name="work", bufs=2) as wpool, \
             tc.tile_pool(name="psum", bufs=2, space="PSUM") as ppool:

            # ---- constants ----
            w1r = cpool.tile([128, 9 * 128], f32r)
            w2r = cpool.tile([128, 9 * 128], f32r)
            for wd, wr in [(w1_d, w1r), (w2_d, w2r)]:
                wstage = cpool.tile([128, 9 * 128], f32, tag="wstage", name="wstage")
                nc.sync.dma_start(wstage[:].rearrange("c (s o) -> c s o", s=9),
                                  wd[:].rearrange("s c o -> c s o"))
                nc.vector.tensor_copy(wr[:], wstage[:])
            gh = cpool.tile([128, 4], f32)
            nc.sync.dma_start(gh[:], gh_d[:])
            bigc = cpool.tile([128, 1], f32)
            nc.vector.memset(bigc[:], BIGC)

            def quant_layer(src_hi, src_lo, wr, K, scales):
                """9-shift quantized conv from padded f32r pair -> K bf16 [128, NPIX]."""
                for g in range(NG):
                    for s in range(9):
                        dh, dw = SHIFTS[s]
                        pg = ppool.tile([128, CPG * 512], f32, name=f"pg")
                        pg3 = pg[:].rearrange("p (b n) -> p b n", b=CPG)
                        for k in range(CPG):
                            r0 = (g * CPG + k) * RPC
                            hi3 = src_hi[:].rearrange("p (h w) -> p h w", h=Hp)
                            lo3 = src_lo[:].rearrange("p (h w) -> p h w", h=Hp)
                            rhs_hi = hi3[:, r0 + dh:r0 + dh + RPC, dw:dw + Wimg]
                            rhs_lo = lo3[:, r0 + dh:r0 + dh + RPC, dw:dw + Wimg]
                            lhsT = wr[:, s * 128:(s + 1) * 128]
                            nc.tensor.matmul(pg3[:, k, 0:NCOL], lhsT, rhs_hi,
                                             start=True, stop=False)
                            nc.tensor.matmul(pg3[:, k, 0:NCOL], lhsT, rhs_lo,
                                             start=False, stop=True)
                        # evac + scale + RNE-round via fp32 magic add
                        t = wpool.tile([128, NGRP], f32, name="t_evac")
                        nc.scalar.activation(t[:].rearrange("p (b n) -> p b n", b=CPG),
                                             pg3[:, :, 0:NCOL], AF.Identity,
                                             bias=bigc[:], scale=scales[s])
                        u = wpool.tile([128, NGRP], bf16, name="u_sub")
                        nc.vector.tensor_scalar(u[:], t[:], BIGC, NBITS_QN,
                                                op0=OP.subtract, op1=OP.max)
                        Ks = K[:, g * NGRP:(g + 1) * NGRP]
                        if s == 0:
                            nc.vector.tensor_scalar(Ks, u[:], NBITS_QP, None,
                                                    op0=OP.min)
                        else:
                            c = wpool.tile([128, NGRP], bf16, name="c_clip")
                            nc.vector.tensor_scalar(c[:], u[:], NBITS_QP, None,
                                                    op0=OP.min)
                            nc.vector.tensor_tensor(Ks, Ks, c[:], op=OP.add)

            def zero_borders(t3):
                nc.vector.memset(t3[:, 0:1, :], 0.0)
                nc.vector.memset(t3[:, Hp - 1:Hp, :], 0.0)
                nc.vector.memset(t3[:, 1:Hp - 1, 0:1], 0.0)
                nc.vector.memset(t3[:, 1:Hp - 1, Wp - 1:Wp], 0.0)

            for i in range(B_loc):
                # ---- load + pad + split x (lo residual written as f32r directly) ----
                xp = ipool.tile([128, NPAD], f32, tag="padA", name="xp")
                xp3 = xp[:].rearrange("p (h w) -> p h w", h=Hp)
                zero_borders(xp3)
                nc.sync.dma_start(xp3[:, 1:Hp - 1, 1:Wp - 1],
                                  x_d[i].rearrange("c (h w) -> c h w", h=Himg))
                x_r = ipool.tile([128, NPAD], f32r, name="x_r")
                nc.vector.tensor_copy(x_r[:], xp[:])
                xlo_r = ipool.tile([128, NPAD], f32r, name="xlo_r")
                nc.vector.tensor_tensor(xlo_r[:], xp[:], x_r[:].bitcast(f32),
                                        op=OP.subtract)

                # ---- layer 1 ----
                K1 = kpool.tile([128, NPIX], bf16, name="K1")
                quant_layer(x_r, xlo_r, w1r, K1, scales1)

                # ---- transition: y = relu(g1*K1 + h1), pad, split ----
                tpad = ipool.tile([128, NPAD], f32, tag="padA", name="tpad")
                tp3 = tpad[:].rearrange("p (h w) -> p h w", h=Hp)
                zero_borders(tp3)
                nc.vector.tensor_scalar(tp3[:, 1:Hp - 1, 1:Wp - 1],
                                        K1[:].rearrange("p (h w) -> p h w", h=Himg),
                                        gh[:, 0:1], gh[:, 1:2],
                                        op0=OP.mult, op1=OP.add)
                yf = ipool.tile([128, NPAD], f32, tag="padB", name="yf")
                nc.vector.tensor_scalar(yf[:], tpad[:], 0.0, None, op0=OP.max)
                y_r = ipool.tile([128, NPAD], f32r, name="y_r")
                nc.vector.tensor_copy(y_r[:], yf[:])
                ylo_r = ipool.tile([128, NPAD], f32r, name="ylo_r")
                nc.vector.tensor_tensor(ylo_r[:], yf[:], y_r[:].bitcast(f32),
                                        op=OP.subtract)

                if debug:
                    k1f = ipool.tile([128, NPIX], f32, name="k1f")
                    nc.vector.tensor_copy(k1f[:], K1[:])
                    nc.sync.dma_start(k1_d[i], k1f[:])
                    nc.sync.dma_start(y_d[i], yf[:])

                # ---- layer 2 ----
                K2 = ipool.tile([128, NPIX], bf16, name="K2")
                quant_layer(y_r, ylo_r, w2r, K2, scales2)

                # ---- final: out = relu(g2*K2 + h2 + x) ----
                xi2 = ipool.tile([128, NPIX], f32, name="xi2")
                nc.sync.dma_start(xi2[:], x_d[i])
                t2 = ipool.tile([128, NPIX], f32, tag="fin", name="t2")
                nc.vector.tensor_scalar(t2[:], K2[:], gh[:, 2:3], gh[:, 3:4],
                                        op0=OP.mult, op1=OP.add)
                ob = ipool.tile([128, NPIX], f32, name="ob")
                nc.vector.tensor_tensor(ob[:], t2[:], xi2[:], op=OP.add)
                o2 = ipool.tile([128, NPIX], f32, tag="fin", name="o2")
                nc.scalar.activation(o2[:], ob[:], AF.Relu)
                nc.sync.dma_start(out_d[i], o2[:])

    nc.compile()
    return nc


def _host_prep(inputs):
    """Quantize weights + fold BN exactly as the fp32 reference does."""
    i = {k: np.asarray(v) for k, v in inputs.items()}
    x = i["x"].astype(np.float32, copy=False)
    outs = {}
    for L, (Wk, awk, apk, g, b, m, v) in enumerate(
        [("W1", "a_w1", "a_p1", "bn1_gamma", "bn1_beta", "bn1_mean", "bn1_var"),
         ("W2", "a_w2", "a_p2", "bn2_gamma", "bn2_beta", "bn2_mean", "bn2_var")],
        start=1,
    ):
        W = i[Wk].astype(np.float32, copy=False)       # [9, O, C]
        a_w = i[awk].astype(np.float32, copy=False)    # [9]
        a_p = np.float32(i[apk])
        Wint = np.round(np.clip(W / a_w[:, None, None], -4.0, 3.0)).astype(np.float32)
        outs[f"w{L}T"] = np.ascontiguousarray(np.transpose(Wint, (0, 2, 1)))  # [9,C,O]
        outs[f"s{L}"] = tuple(float(np.float32(aw) / a_p) for aw in a_w)
        inv = i[g].astype(np.float32) / np.sqrt(i[v].astype(np.float32) + np.float32(1e-5))
        outs[f"g{L}"] = (a_p * inv).astype(np.float32)
        outs[f"h{L}"] = (i[b].astype(np.float32) - i[m].astype(np.float32) * inv).astype(np.float32)
    outs["x"] = x
    return outs


def kernel(**inputs):
    from concourse.bass_utils import run_bass_kernel_spmd

    p = _host_prep(inputs)
    x = p["x"]
    B, C, H, W = x.shape
    n_cores = 8
    B_loc = B // n_cores

    key = (B_loc, H, W, p["s1"], p["s2"])
    if key not in _CACHE:
        _CACHE[key] = _build(B_loc, H, W, p["s1"], p["s2"])
    nc = _CACHE[key]

    gh = np.stack([p["g1"], p["h1"], p["g2"], p["h2"]], axis=1).astype(np.float32)
    xs = x.reshape(n_cores, B_loc, C, H * W)
    in_maps = [{"x": np.ascontiguousarray(xs[c]), "w1": p["w1T"], "w2": p["w2T"],
                "gh": gh} for c in range(n_cores)]
    res = run_bass_kernel_spmd(nc, in_maps, core_ids=list(range(n_cores)))
    out = np.concatenate([r["out"][None] for r in res.results], axis=0)
    return out.reshape(B, C, H, W).astype(np.float32, copy=False)
